# revision 1
# baseline (speedup 1.0000x reference)
"""Trainium2 Bass kernel for DyDepthwiseConvAtten.

Computation (per (b, n) row r of C=256 channels):
  w[r, k]  = sum_c q[r, c] * W_w[k, c] + b_w[k]          (k = 0..2)
  x[r, c]  = sum_k w[r, k] * vpad[r, c + k]               (3-tap depthwise conv, 'same')
  out[r,c] = (x - mean_c(x)) * rsqrt(var_c(x) + eps) * gamma[c] + beta[c]

Strategy: pure data-parallel over batch across 8 cores. Rows (b*n flattened)
live on SBUF partitions; tiles of 128 rows x 256 channels.

  - w via TensorE: host pre-transposes q so each tile's qT chunk [128c, 128r]
    is the (self-loading, fp32-exact) stationary operand against W_w^T [128c, 3].
  - conv via TensorE: x_psum += diag(w_k) @ v_shifted_k for k = 0..2, where
    diag(w_k) = identity * w_k is built with one per-partition tensor_scalar
    each.  float32r mode streams 1 column/cycle (4x faster than fp32).
  - LayerNorm: bn_stats/bn_aggr on VectorE, sqrt on ScalarE, reciprocal on
    VectorE, and the normalize as one ScalarE activation
    y = Identity(x * rs + (-mu * rs)).
"""

import os
from contextlib import ExitStack

import numpy as np

import concourse.bacc as bacc
import concourse.bass as bass
import concourse.tile as tile
from concourse import mybir
from concourse.bass_utils import run_bass_kernel_spmd
from concourse.masks import make_identity

B, N, C, K = 1024, 100, 256, 3
N_CORES = 8
B_PER_CORE = B // N_CORES        # 128
ROWS = B_PER_CORE * N            # 12800 rows per core
P = 128                          # partitions (rows per tile)
N_ROW_TILES = ROWS // P          # 100
LN_EPS = 1e-5
F32 = mybir.dt.float32
F32R = mybir.dt.float32r

# conv matmul precision mode: "f32r" (fast, reduced-precision multiply),
# "f32" (exact, 4 cycles/row), "dve" (exact, vector-engine conv)
CONV_MODE = os.environ.get("BASS_DYCONV_MODE", "f32r")
TRACE = bool(int(os.environ.get("BASS_DYCONV_TRACE", "0")))

LAST_EXEC_NS = None
LAST_RESULTS = None

_cache = {}


def _build(conv_mode: str, apply_affine: bool, add_bias: bool):
    nc = bacc.Bacc("TRN2", target_bir_lowering=False, debug=False)

    # qT: per-core transposed query, chunked: [2, 128c, ROWS]
    qT = nc.dram_tensor("qt", [2, P, ROWS], F32, kind="ExternalInput")
    v = nc.dram_tensor("v", [ROWS, C], F32, kind="ExternalInput")
    # W_w^T chunks: [2, 128c, K]
    wwt = nc.dram_tensor("wwt", [2, P, K], F32, kind="ExternalInput")
    out = nc.dram_tensor("out", [ROWS, C], F32, kind="ExternalOutput")
    gamma = beta = bwb = None
    if apply_affine:
        gamma = nc.dram_tensor("gamma", [1, C], F32, kind="ExternalInput")
        beta = nc.dram_tensor("beta", [1, C], F32, kind="ExternalInput")
    if add_bias:
        bwb = nc.dram_tensor("bw", [1, K], F32, kind="ExternalInput")

    with tile.TileContext(nc) as tc, ExitStack() as ctx:
        _emit(ctx, tc, qT.ap(), v.ap(), wwt.ap(), out.ap(),
              gamma.ap() if gamma is not None else None,
              beta.ap() if beta is not None else None,
              bwb.ap() if bwb is not None else None,
              conv_mode)
    nc.compile()
    return nc


def _bcast_rows(ap: bass.AP, nrows: int) -> bass.AP:
    """DMA access pattern replicating a [1, F] DRAM tensor across partitions."""
    return bass.AP(tensor=ap.tensor, offset=ap.offset,
                   ap=[[0, nrows]] + list(ap.ap[1:]))


def _emit(ctx, tc, qT, v, wwt, out, gamma, beta, bwb, conv_mode):
    nc = tc.nc
    mult = mybir.AluOpType.mult
    AF = mybir.ActivationFunctionType

    singles = ctx.enter_context(tc.tile_pool(name="singles", bufs=1))
    pool = ctx.enter_context(tc.tile_pool(name="work", bufs=4))
    small = ctx.enter_context(tc.tile_pool(name="small", bufs=4))
    psum_x = ctx.enter_context(
        tc.tile_pool(name="psum_x", bufs=3, space=bass.MemorySpace.PSUM))
    psum_w = ctx.enter_context(
        tc.tile_pool(name="psum_w", bufs=2, space=bass.MemorySpace.PSUM))

    # one-time constants
    ident = singles.tile([P, P], F32)
    make_identity(nc, ident[:])
    wwt_sb = singles.tile([P, 2, K], F32)
    nc.sync.dma_start(out=wwt_sb[:], in_=wwt.rearrange("a p k -> p a k"))
    eps_sb = singles.tile([P, 1], F32)
    nc.vector.memset(eps_sb[:], LN_EPS)
    if gamma is not None:
        gamma_sb = singles.tile([P, C], F32)
        nc.sync.dma_start(out=gamma_sb[:], in_=_bcast_rows(gamma, P))
        beta_sb = singles.tile([P, C], F32)
        nc.sync.dma_start(out=beta_sb[:], in_=_bcast_rows(beta, P))
    if bwb is not None:
        bw_sb = singles.tile([P, K], F32)
        nc.sync.dma_start(out=bw_sb[:], in_=_bcast_rows(bwb, P))

    for i in range(N_ROW_TILES):
        r0 = i * P

        # ---- loads ----
        qt_t = pool.tile([P, 2, P], F32, tag="qt")
        nc.sync.dma_start(out=qt_t[:],
                          in_=qT[:, :, r0:r0 + P].rearrange("a c r -> c a r"))
        v_t = pool.tile([P, C + 2], F32, tag="vt")
        nc.gpsimd.memset(v_t[:, 0:1], 0.0)
        nc.gpsimd.memset(v_t[:, C + 1:C + 2], 0.0)
        nc.sync.dma_start(out=v_t[:, 1:C + 1], in_=v[r0:r0 + P, :])

        # ---- dynamic weights w[r, k] on TensorE (fp32 exact) ----
        w_ps = psum_w.tile([P, K], F32, tag="w_ps")
        nc.tensor.matmul(w_ps[:], lhsT=qt_t[:, 0, :], rhs=wwt_sb[:, 0, :],
                         start=True, stop=False)
        nc.tensor.matmul(w_ps[:], lhsT=qt_t[:, 1, :], rhs=wwt_sb[:, 1, :],
                         start=False, stop=True)
        w_sb = small.tile([P, K], F32, tag="w_sb")
        if bwb is not None:
            nc.vector.tensor_add(w_sb[:], w_ps[:], bw_sb[:])
        else:
            nc.scalar.copy(w_sb[:], w_ps[:])

        # ---- 3-tap depthwise conv ----
        if conv_mode in ("f32r", "f32"):
            dk = pool.tile([P, K, P], F32, tag="dk")
            for k in range(K):
                nc.vector.tensor_scalar_mul(dk[:, k, :], ident[:],
                                            w_sb[:, k:k + 1])
            x_ps = psum_x.tile([P, C], F32, tag="x_ps")
            for k in range(K):
                lhsT = dk[:, k, :]
                rhs = v_t[:, k:k + C]
                if conv_mode == "f32r":
                    lhsT = lhsT.bitcast(F32R)
                    rhs = rhs.bitcast(F32R)
                nc.tensor.matmul(x_ps[:], lhsT=lhsT, rhs=rhs,
                                 start=(k == 0), stop=(k == K - 1))
            x = x_ps
        else:  # "dve": exact conv on VectorE/ScalarE
            t0 = pool.tile([P, C], F32, tag="t0")
            nc.vector.tensor_scalar_mul(t0[:], v_t[:, 0:C], w_sb[:, 0:1])
            t1 = pool.tile([P, C], F32, tag="t1")
            nc.scalar.activation(t1[:], v_t[:, 1:C + 1], AF.Copy,
                                 scale=w_sb[:, 1:2])
            t2 = pool.tile([P, C], F32, tag="t2")
            nc.vector.tensor_scalar_mul(t2[:], v_t[:, 2:C + 2], w_sb[:, 2:3])
            x01 = pool.tile([P, C], F32, tag="x01")
            nc.vector.tensor_add(x01[:], t0[:], t1[:])
            x_sb = pool.tile([P, C], F32, tag="x_sb")
            nc.vector.tensor_add(x_sb[:], x01[:], t2[:])
            x = x_sb

        # ---- LayerNorm over channels ----
        stats = small.tile([P, 6], F32, tag="stats")
        nc.vector.bn_stats(out=stats[:], in_=x[:])
        mv = small.tile([P, 2], F32, tag="mv")
        nc.vector.bn_aggr(out=mv[:], in_=stats[:])
        std = small.tile([P, 1], F32, tag="std")
        nc.scalar.activation(std[:], mv[:, 1:2], AF.Sqrt, bias=eps_sb[:])
        rs = small.tile([P, 1], F32, tag="rs")
        nc.vector.reciprocal(rs[:], std[:])
        nbias = small.tile([P, 1], F32, tag="nbias")
        nc.vector.tensor_scalar(out=nbias[:], in0=mv[:, 0:1], scalar1=rs[:],
                                scalar2=-1.0, op0=mult, op1=mult)
        y = pool.tile([P, C], F32, tag="y")
        nc.scalar.activation(y[:], x[:], AF.Identity, bias=nbias[:],
                             scale=rs[:])
        if gamma is not None:
            nc.vector.tensor_mul(y[:], y[:], gamma_sb[:])
            nc.vector.tensor_add(y[:], y[:], beta_sb[:])

        nc.sync.dma_start(out=out[r0:r0 + P, :], in_=y[:])


def kernel(query, value, W_w, b_w, gamma, beta):
    global LAST_EXEC_NS, LAST_RESULTS

    query = np.ascontiguousarray(np.asarray(query, dtype=np.float32))
    value = np.ascontiguousarray(np.asarray(value, dtype=np.float32))
    W_w = np.ascontiguousarray(np.asarray(W_w, dtype=np.float32))
    b_w = np.asarray(b_w, dtype=np.float32)
    gamma = np.asarray(gamma, dtype=np.float32)
    beta = np.asarray(beta, dtype=np.float32)

    apply_affine = not (np.all(gamma == 1.0) and np.all(beta == 0.0))
    add_bias = bool(np.any(b_w != 0.0))

    key = (CONV_MODE, apply_affine, add_bias)
    if key not in _cache:
        _cache[key] = _build(*key)
    nc = _cache[key]

    # host-side layout prep (sharding + transposes, no flops)
    wwt = np.ascontiguousarray(W_w.T.reshape(2, P, K))
    q_sh = query.reshape(N_CORES, ROWS, C)
    v_sh = value.reshape(N_CORES, ROWS, C)

    in_maps = []
    for c in range(N_CORES):
        m = {
            "qt": np.ascontiguousarray(q_sh[c].T).reshape(2, P, ROWS),
            "v": v_sh[c],
            "wwt": wwt,
        }
        if apply_affine:
            m["gamma"] = gamma.reshape(1, C)
            m["beta"] = beta.reshape(1, C)
        if add_bias:
            m["bw"] = b_w.reshape(1, K)
        in_maps.append(m)

    res = run_bass_kernel_spmd(nc, in_maps, core_ids=list(range(N_CORES)),
                               trace=TRACE)
    LAST_EXEC_NS = res.exec_time_ns
    LAST_RESULTS = res
    out = np.empty((B, N, C), dtype=np.float32)
    for c in range(N_CORES):
        out[c * B_PER_CORE:(c + 1) * B_PER_CORE] = (
            res.results[c]["out"].reshape(B_PER_CORE, N, C))
    return out


# revision 3
# speedup vs baseline: 283.8241x; 283.8241x over previous
"""Trainium2 Bass kernel for DyDepthwiseConvAtten.

Computation (per (b, n) row r of C=256 channels):
  w[r, k]  = sum_c q[r, c] * W_w[k, c] + b_w[k]          (k = 0..2)
  x[r, c]  = sum_k w[r, k] * vpad[r, c + k]               (3-tap depthwise conv, 'same')
  out[r,c] = (x - mean_c(x)) * rsqrt(var_c(x) + eps) * gamma[c] + beta[c]

Strategy: pure data-parallel over batch across 8 cores. Rows (b*n flattened)
live on SBUF partitions; tiles of 128 rows x 256 channels.

  - w via TensorE: host pre-transposes q so each tile's qT chunk [128c, 128r]
    is the (self-loading, fp32-exact) stationary operand against W_w^T [128c, 3].
  - conv via TensorE: x_psum += diag(w_k) @ v_shifted_k for k = 0..2, where
    diag(w_k) = identity * w_k is built with one per-partition tensor_scalar
    each.  float32r mode streams 1 column/cycle (4x faster than fp32).
  - LayerNorm: bn_stats/bn_aggr on VectorE, sqrt on ScalarE, reciprocal on
    VectorE, and the normalize as one ScalarE activation
    y = Identity(x * rs + (-mu * rs)).
"""

import os
from contextlib import ExitStack

import numpy as np

import concourse.bacc as bacc
import concourse.bass as bass
import concourse.tile as tile
from concourse import mybir
from concourse.bass_utils import run_bass_kernel_spmd
from concourse.masks import make_identity

B, N, C, K = 1024, 100, 256, 3
N_CORES = 8
B_PER_CORE = B // N_CORES        # 128
ROWS = B_PER_CORE * N            # 12800 rows per core
P = 128                          # partitions (rows per tile)
N_ROW_TILES = ROWS // P          # 100
LN_EPS = 1e-5
F32 = mybir.dt.float32
F32R = mybir.dt.float32r

# conv matmul precision mode: "f32r" (fast, reduced-precision multiply),
# "f32" (exact, 4 cycles/row), "dve" (exact, vector-engine conv)
CONV_MODE = os.environ.get("BASS_DYCONV_MODE", "f32r")
TRACE = bool(int(os.environ.get("BASS_DYCONV_TRACE", "0")))

LAST_EXEC_NS = None
LAST_RESULTS = None

_cache = {}


def _build(conv_mode: str, apply_affine: bool, add_bias: bool,
           loop_n: int = 1):
    nc = bacc.Bacc("TRN2", target_bir_lowering=False, debug=False)

    # qT: per-core transposed query, chunked: [2, 128c, ROWS]
    qT = nc.dram_tensor("qt", [2, P, ROWS], F32, kind="ExternalInput")
    v = nc.dram_tensor("v", [ROWS, C], F32, kind="ExternalInput")
    # W_w^T chunks: [2, 128c, K]
    wwt = nc.dram_tensor("wwt", [2, P, K], F32, kind="ExternalInput")
    out = nc.dram_tensor("out", [ROWS, C], F32, kind="ExternalOutput")
    gamma = beta = bwb = None
    if apply_affine:
        gamma = nc.dram_tensor("gamma", [1, C], F32, kind="ExternalInput")
        beta = nc.dram_tensor("beta", [1, C], F32, kind="ExternalInput")
    if add_bias:
        bwb = nc.dram_tensor("bw", [1, K], F32, kind="ExternalInput")

    with tile.TileContext(nc) as tc, ExitStack() as ctx:
        if loop_n > 1:
            with tc.For_i(0, loop_n, 1):
                _emit(ctx, tc, qT.ap(), v.ap(), wwt.ap(), out.ap(),
                      gamma.ap() if gamma is not None else None,
                      beta.ap() if beta is not None else None,
                      bwb.ap() if bwb is not None else None,
                      conv_mode)
        else:
            _emit(ctx, tc, qT.ap(), v.ap(), wwt.ap(), out.ap(),
                  gamma.ap() if gamma is not None else None,
                  beta.ap() if beta is not None else None,
                  bwb.ap() if bwb is not None else None,
                  conv_mode)
    nc.compile()
    return nc


def _bcast_rows(ap: bass.AP, nrows: int) -> bass.AP:
    """DMA access pattern replicating a [1, F] DRAM tensor across partitions."""
    return bass.AP(tensor=ap.tensor, offset=ap.offset,
                   ap=[[0, nrows]] + list(ap.ap[1:]))


def _emit(ctx, tc, qT, v, wwt, out, gamma, beta, bwb, conv_mode):
    nc = tc.nc
    mult = mybir.AluOpType.mult
    AF = mybir.ActivationFunctionType

    singles = ctx.enter_context(tc.tile_pool(name="singles", bufs=1))
    pool = ctx.enter_context(tc.tile_pool(name="work", bufs=4))
    small = ctx.enter_context(tc.tile_pool(name="small", bufs=4))
    psum_x = ctx.enter_context(
        tc.tile_pool(name="psum_x", bufs=3, space=bass.MemorySpace.PSUM))
    psum_w = ctx.enter_context(
        tc.tile_pool(name="psum_w", bufs=2, space=bass.MemorySpace.PSUM))

    # one-time constants
    ident = singles.tile([P, P], F32)
    make_identity(nc, ident[:])
    wwt_sb = singles.tile([P, 2, K], F32)
    nc.sync.dma_start(out=wwt_sb[:], in_=wwt.rearrange("a p k -> p a k"))
    eps_sb = singles.tile([P, 1], F32)
    nc.vector.memset(eps_sb[:], LN_EPS)
    if gamma is not None:
        gamma_sb = singles.tile([P, C], F32)
        nc.sync.dma_start(out=gamma_sb[:], in_=_bcast_rows(gamma, P))
        beta_sb = singles.tile([P, C], F32)
        nc.sync.dma_start(out=beta_sb[:], in_=_bcast_rows(beta, P))
    if bwb is not None:
        bw_sb = singles.tile([P, K], F32)
        nc.sync.dma_start(out=bw_sb[:], in_=_bcast_rows(bwb, P))

    for i in range(N_ROW_TILES):
        r0 = i * P

        # ---- loads ----
        qt_t = pool.tile([P, 2, P], F32, tag="qt")
        nc.sync.dma_start(out=qt_t[:],
                          in_=qT[:, :, r0:r0 + P].rearrange("a c r -> c a r"))
        v_t = pool.tile([P, C + 2], F32, tag="vt")
        nc.gpsimd.memset(v_t[:, 0:1], 0.0)
        nc.gpsimd.memset(v_t[:, C + 1:C + 2], 0.0)
        nc.sync.dma_start(out=v_t[:, 1:C + 1], in_=v[r0:r0 + P, :])

        # ---- dynamic weights w[r, k] on TensorE (fp32 exact) ----
        w_ps = psum_w.tile([P, K], F32, tag="w_ps")
        nc.tensor.matmul(w_ps[:], lhsT=qt_t[:, 0, :], rhs=wwt_sb[:, 0, :],
                         start=True, stop=False)
        nc.tensor.matmul(w_ps[:], lhsT=qt_t[:, 1, :], rhs=wwt_sb[:, 1, :],
                         start=False, stop=True)
        w_sb = small.tile([P, K], F32, tag="w_sb")
        if bwb is not None:
            nc.vector.tensor_add(w_sb[:], w_ps[:], bw_sb[:])
        else:
            nc.scalar.copy(w_sb[:], w_ps[:])

        # ---- 3-tap depthwise conv ----
        if conv_mode in ("f32r", "f32"):
            dk = pool.tile([P, K, P], F32, tag="dk")
            for k in range(K):
                nc.vector.tensor_scalar_mul(dk[:, k, :], ident[:],
                                            w_sb[:, k:k + 1])
            x_ps = psum_x.tile([P, C], F32, tag="x_ps")
            for k in range(K):
                lhsT = dk[:, k, :]
                rhs = v_t[:, k:k + C]
                if conv_mode == "f32r":
                    lhsT = lhsT.bitcast(F32R)
                    rhs = rhs.bitcast(F32R)
                nc.tensor.matmul(x_ps[:], lhsT=lhsT, rhs=rhs,
                                 start=(k == 0), stop=(k == K - 1))
            x = x_ps
        else:  # "dve": exact conv on VectorE/ScalarE
            t0 = pool.tile([P, C], F32, tag="t0")
            nc.vector.tensor_scalar_mul(t0[:], v_t[:, 0:C], w_sb[:, 0:1])
            t1 = pool.tile([P, C], F32, tag="t1")
            nc.scalar.activation(t1[:], v_t[:, 1:C + 1], AF.Copy,
                                 scale=w_sb[:, 1:2])
            t2 = pool.tile([P, C], F32, tag="t2")
            nc.vector.tensor_scalar_mul(t2[:], v_t[:, 2:C + 2], w_sb[:, 2:3])
            x01 = pool.tile([P, C], F32, tag="x01")
            nc.vector.tensor_add(x01[:], t0[:], t1[:])
            x_sb = pool.tile([P, C], F32, tag="x_sb")
            nc.vector.tensor_add(x_sb[:], x01[:], t2[:])
            x = x_sb

        # ---- LayerNorm over channels ----
        stats = small.tile([P, 6], F32, tag="stats")
        nc.vector.bn_stats(out=stats[:], in_=x[:])
        mv = small.tile([P, 2], F32, tag="mv")
        nc.vector.bn_aggr(out=mv[:], in_=stats[:])
        std = small.tile([P, 1], F32, tag="std")
        nc.scalar.activation(std[:], mv[:, 1:2], AF.Sqrt, bias=eps_sb[:])
        rs = small.tile([P, 1], F32, tag="rs")
        nc.vector.reciprocal(rs[:], std[:])
        nbias = small.tile([P, 1], F32, tag="nbias")
        nc.vector.tensor_scalar(out=nbias[:], in0=mv[:, 0:1], scalar1=rs[:],
                                scalar2=-1.0, op0=mult, op1=mult)
        y = pool.tile([P, C], F32, tag="y")
        nc.scalar.activation(y[:], x[:], AF.Identity, bias=nbias[:],
                             scale=rs[:])
        if gamma is not None:
            nc.vector.tensor_mul(y[:], y[:], gamma_sb[:])
            nc.vector.tensor_add(y[:], y[:], beta_sb[:])

        nc.sync.dma_start(out=out[r0:r0 + P, :], in_=y[:])


def kernel(query, value, W_w, b_w, gamma, beta):
    global LAST_EXEC_NS, LAST_RESULTS

    query = np.ascontiguousarray(np.asarray(query, dtype=np.float32))
    value = np.ascontiguousarray(np.asarray(value, dtype=np.float32))
    W_w = np.ascontiguousarray(np.asarray(W_w, dtype=np.float32))
    b_w = np.asarray(b_w, dtype=np.float32)
    gamma = np.asarray(gamma, dtype=np.float32)
    beta = np.asarray(beta, dtype=np.float32)

    apply_affine = not (np.all(gamma == 1.0) and np.all(beta == 0.0))
    add_bias = bool(np.any(b_w != 0.0))

    key = (CONV_MODE, apply_affine, add_bias)
    if key not in _cache:
        _cache[key] = _build(*key)
    nc = _cache[key]

    # host-side layout prep (sharding + transposes, no flops)
    wwt = np.ascontiguousarray(W_w.T.reshape(2, P, K))
    q_sh = query.reshape(N_CORES, ROWS, C)
    v_sh = value.reshape(N_CORES, ROWS, C)

    in_maps = []
    for c in range(N_CORES):
        m = {
            "qt": np.ascontiguousarray(q_sh[c].T).reshape(2, P, ROWS),
            "v": v_sh[c],
            "wwt": wwt,
        }
        if apply_affine:
            m["gamma"] = gamma.reshape(1, C)
            m["beta"] = beta.reshape(1, C)
        if add_bias:
            m["bw"] = b_w.reshape(1, K)
        in_maps.append(m)

    res = run_bass_kernel_spmd(nc, in_maps, core_ids=list(range(N_CORES)),
                               trace=TRACE)
    LAST_EXEC_NS = res.exec_time_ns
    LAST_RESULTS = res
    out = np.empty((B, N, C), dtype=np.float32)
    for c in range(N_CORES):
        out[c * B_PER_CORE:(c + 1) * B_PER_CORE] = (
            res.results[c]["out"].reshape(B_PER_CORE, N, C))
    return out
